# revision 33
# baseline (speedup 1.0000x reference)
"""Trainium2 Bass kernel for nn_BaseLUTLayer (soft-LUT layer).

Math: out[b,o] = sum_k lut[o,k] * prod_j (bit_j(k) ? x[b,m(o,j)] : 1-x[b,m(o,j)])

Strategy (per core, batch-sharded 8 ways, 128 batch rows each):
  * odds transform: with w = 1-x, r = x/(1-x):
        out[b,o] = (prod_j w_j) * H,   H = successive halving of lut with
        T_new[k'] = T_lo[k'] + r_j * T_hi[k']   (6 levels, 2 DVE ops/elem)
  * layout: nodes on SBUF partitions (o_p = o % 128), free dims (k', b).
    lut tiles live per-partition (no replication); r/w values are gathered
    per (node, wire) with dma_gather using compile-time indices derived
    from `mapping` (host-known at trace time).
  * gather source: G[row(i)] = [w[:,i] (128 f32) | r[:,i] (128 f32)] built
    on-device (clamp, 1-x, reciprocal, PE transposes) and bounced via HBM.
"""

import numpy as np

import concourse.bass as bass
import concourse.mybir as mybir
from concourse import bacc
from concourse import tile
from concourse.masks import make_identity
from concourse.bass_utils import run_bass_kernel_spmd

P = 128
IN = 1024
OUT = 2048
NB = 6
B_FULL = 1024
N_CORES = 8
OHI = OUT // P  # 16
F32 = mybir.dt.float32
I16 = mybir.dt.int16
# clamp x <= 1 - 2^-18 so r = x/(1-x) <= 2^18 and r^6 stays well inside fp32
CLAMP = float(1.0 - 2.0**-18)

# chunks of the o_hi loop assigned to gpsimd instead of DVE (load balance:
# gpsimd 2-input elementwise is ~2x slower than DVE, so give it ~1/3)
GPSIMD_CHUNKS = ()


def _mult():
    return mybir.AluOpType.mult


def _add():
    return mybir.AluOpType.add


def build_program():
    nc = bacc.Bacc("TRN2", target_bir_lowering=False, debug=False)

    xs = nc.dram_tensor("xs", [P, IN], F32, kind="ExternalInput").ap()
    gidx = nc.dram_tensor("gidx", [P, OUT * NB // 16], I16, kind="ExternalInput").ap()
    lutg = nc.dram_tensor("lutg", [P, OHI, 64], F32, kind="ExternalInput").ap()
    outs = nc.dram_tensor("outs", [P, OHI, P], F32, kind="ExternalOutput").ap()

    with tile.TileContext(nc) as tc:
        with (
            tc.tile_pool(name="consts", bufs=1) as consts,
            tc.tile_pool(name="main", bufs=1) as main,
            tc.tile_pool(name="zpool", bufs=4) as zpool,
            tc.tile_pool(name="tpool", bufs=3) as tpool,
            tc.tile_pool(name="spool", bufs=2) as spool,
            tc.tile_pool(name="dram", bufs=1, space="DRAM") as dpool,
        ):
            ident = consts.tile([P, P], F32)
            make_identity(nc, ident)

            gd = dpool.tile([P * (IN // P), 2 * P], F32)
            gd_warm = gd

            gidx_sb = consts.tile([P, OUT * NB // 16], I16)
            nc.sync.dma_start(gidx_sb, gidx)
            lutg_sb = consts.tile([P, OHI, 64], F32)
            nc.sync.dma_start(lutg_sb, lutg)

            # warm up the dma_gather ucode (IRAM load) before G is ready:
            # zero gd row 0, gather it 128 times into a scratch tile
            wzt = consts.tile([1, 2 * P], F32)
            nc.gpsimd.memset(wzt, 0.0)
            nc.sync.dma_start(gd_warm[0:1, :], wzt)
            widx = consts.tile([P, 8], I16)
            nc.gpsimd.memset(widx, 0)
            warm = consts.tile([P, 1, 2 * P], F32)
            nc.gpsimd.dma_gather(
                out_ap=warm,
                in_ap=gd_warm[0:1, :],
                idxs_ap=widx,
                num_idxs=P,
                num_idxs_reg=P,
                elem_size=2 * P,
            )

            # x shard, clamped; w = 1-x; r = x * (1/w)
            xt = main.tile([P, IN], F32)
            nc.sync.dma_start(xt, xs)
            nc.vector.tensor_scalar_min(xt, xt, CLAMP)
            wt = main.tile([P, IN], F32)
            nc.vector.tensor_scalar(
                out=wt, in0=xt, scalar1=-1.0, scalar2=1.0, op0=_mult(), op1=_add()
            )
            rw = main.tile([P, IN], F32)
            rt = main.tile([P, IN], F32)
            for q in range(4):
                qs = slice(q * (IN // 4), (q + 1) * (IN // 4))
                nc.vector.reciprocal(rw[:, qs], wt[:, qs])
                nc.vector.tensor_mul(rt[:, qs], xt[:, qs], rw[:, qs])

            # transpose w/r into G rows: G[(i%128)*8 + i//128] = [w[:,i] | r[:,i]]
            gsb = main.tile([P, IN // P, 2 * P], F32)
            with tc.tile_pool(name="psum_t", bufs=2, space="PSUM") as psum_t:
                for ih in range(IN // P):
                    pw = psum_t.tile([P, P], F32, tag="pt")
                    nc.tensor.transpose(pw, wt[:, ih * P : (ih + 1) * P], ident)
                    nc.scalar.copy(gsb[:, ih, 0:P], pw)
                    pr = psum_t.tile([P, P], F32, tag="pt")
                    nc.tensor.transpose(pr, rt[:, ih * P : (ih + 1) * P], ident)
                    nc.scalar.copy(gsb[:, ih, P : 2 * P], pr)

            gd_view = gd[:].rearrange("(p h) e -> p h e", h=IN // P)
            nc.sync.dma_start(gd_view, gsb)

            # main loop over node chunks (128 nodes each)
            psum_cm = tc.tile_pool(name="psum", bufs=2, space="PSUM")
            psum = psum_cm.__enter__()
            idx_cols = NB * P // 16  # 48 idx columns per chunk

            # two-stage software pipeline: stage A (gather + monomial muls +
            # DMA pair-adds) for chunk c, then stage B (everything after the
            # DMA-adds) for chunk c-1 — keeps DVE's in-order queue from
            # stalling on the DMA-add completion.
            stash = {}

            def stage_a(c):
                z = zpool.tile([P, NB, 2 * P], F32, tag="z")
                nc.gpsimd.dma_gather(
                    out_ap=z,
                    in_ap=gd[:],
                    idxs_ap=gidx_sb[:, c * idx_cols : (c + 1) * idx_cols],
                    num_idxs=NB * P,
                    num_idxs_reg=NB * P,
                    elem_size=2 * P,
                )
                # W = prod_j w_j on GPSIMD — depends only on z (same queue
                # as the gather, so no cross-engine waits on its inputs)
                wp = spool.tile([P, 3, P], F32, tag="wp")
                nc.gpsimd.tensor_mul(wp, z[:, 1:6:2, 0:P], z[:, 0:5:2, 0:P])
                wq = spool.tile([P, P], F32, tag="wq")
                nc.gpsimd.tensor_mul(wq, wp[:, 0, :], wp[:, 1, :])
                nc.gpsimd.tensor_mul(wq, wq, wp[:, 2, :])
                # level 1 (DVE): T1[k'] = lutc0|lutc5 tables: lut lo/hi halves
                t1 = tpool.tile([P, 32, P], F32, tag="t1")
                nc.vector.tensor_mul(
                    t1,
                    z[:, 5, P : 2 * P][:, None, :].broadcast_to([P, 32, P]),
                    lutg_sb[:, c, 32:64][:, :, None].broadcast_to([P, 32, P]),
                )
                nc.vector.tensor_add(
                    t1, t1, lutg_sb[:, c, 0:32][:, :, None].broadcast_to([P, 32, P])
                )
                # level 2 prod (DVE): prod2 = r4 * T1_hi
                prod2 = tpool.tile([P, 16, P], F32, tag="pr16")
                nc.vector.tensor_mul(
                    prod2,
                    z[:, 4, P : 2 * P][:, None, :].broadcast_to([P, 16, P]),
                    t1[:, 16:32, :],
                )
                # L2 on TensorE: acc[0:2048] = T1_lo + prod2 (identity MMs);
                # close regions [1024:2048] (R2,R3) — read-only afterwards
                acc = psum.tile([P, 16 * P], F32, tag="pacc")
                t1f = t1[:].rearrange("p a b -> p (a b)")
                p2f = prod2[:].rearrange("p a b -> p (a b)")
                for s in range(4):
                    sl = slice(s * 512, (s + 1) * 512)
                    nc.tensor.matmul(acc[:, sl], ident, t1f[:, sl], start=True, stop=False)
                    nc.tensor.matmul(acc[:, sl], ident, p2f[:, sl], start=False, stop=(s >= 2))
                stash[c] = (z, acc, wq)

            def stage_b(c):
                z, acc, wq = stash.pop(c)

                # level 3 (j=3, h=8): prod3 = r3*acc[8:16] (DVE, PSUM src);
                # acc[0:8] += prod3 (PE), closing R0,R1
                pn8 = tpool.tile([P, 8, P], F32, tag="pr8")
                nc.vector.tensor_mul(
                    pn8,
                    z[:, 3, P : 2 * P][:, None, :].broadcast_to([P, 8, P]),
                    acc[:, 8 * P : 16 * P].rearrange("p (a b) -> p a b", b=P),
                )
                pn8f = pn8[:].rearrange("p a b -> p (a b)")
                for s in range(2):
                    sl = slice(s * 512, (s + 1) * 512)
                    nc.tensor.matmul(acc[:, sl], ident, pn8f[:, sl], start=False, stop=True)

                # level 4 (j=2, h=4) on DVE from closed PSUM
                pn4 = tpool.tile([P, 4, P], F32, tag="pr4")
                nc.vector.tensor_mul(
                    pn4,
                    z[:, 2, P : 2 * P][:, None, :].broadcast_to([P, 4, P]),
                    acc[:, 4 * P : 8 * P].rearrange("p (a b) -> p a b", b=P),
                )
                t4 = tpool.tile([P, 4, P], F32, tag="t4")
                nc.vector.tensor_add(
                    t4, pn4, acc[:, 0 : 4 * P].rearrange("p (a b) -> p a b", b=P)
                )

                # level 5 (j=1, h=2)
                pn2 = tpool.tile([P, 2, P], F32, tag="pr2")
                nc.vector.tensor_mul(
                    pn2,
                    z[:, 1, P : 2 * P][:, None, :].broadcast_to([P, 2, P]),
                    t4[:, 2:4, :],
                )
                t5 = tpool.tile([P, 2, P], F32, tag="t5")
                nc.vector.tensor_add(t5, pn2, t4[:, 0:2, :])

                # level 6 (j=0, h=1)
                pn1 = tpool.tile([P, 1, P], F32, tag="pr1")
                nc.vector.tensor_mul(
                    pn1,
                    z[:, 0, P : 2 * P][:, None, :].broadcast_to([P, 1, P]),
                    t5[:, 1:2, :],
                )
                t6 = tpool.tile([P, 1, P], F32, tag="t6")
                nc.vector.tensor_add(t6, pn1, t5[:, 0:1, :])

                ot = spool.tile([P, P], F32, tag="ot")
                nc.vector.tensor_mul(ot, t6[:, 0, :], wq)
                nc.sync.dma_start(outs[:, c, :], ot)

            for c in range(OHI + 1):
                if c < OHI:
                    stage_a(c)
                if c >= 1:
                    stage_b(c - 1)
            psum_cm.__exit__(None, None, None)

    # Bacc passes: event-sem generation (multi-wait lowering), auto library
    # loads for dma_gather, extended-InstISA byte packing, ...
    nc.compile()
    return nc


_CACHE: dict = {}


def _program():
    if "nc" not in _CACHE:
        _CACHE["nc"] = build_program()
    return _CACHE["nc"]


def make_inputs(x, lut_table, mapping):
    """Host-side input prep: shard x by batch, encode mapping as gather
    indices, split lut into node-on-partition lo/hi tiles."""
    x = np.ascontiguousarray(x, dtype=np.float32)
    lut_table = np.ascontiguousarray(lut_table, dtype=np.float32)
    mapping = np.asarray(mapping)

    # gather row of source column i: G row (i%128)*8 + i//128
    m3 = mapping.reshape(OHI, P, NB)  # [o_hi, o_p, j]
    rows = (m3 % P) * (IN // P) + (m3 // P)
    # t = (o_hi*NB + j)*128 + o_p  ->  order (o_hi, j, o_p)
    tvals = np.transpose(rows, (0, 2, 1)).reshape(-1)
    gidx16 = tvals.reshape(-1, 16).T.astype(np.int16)  # [16, OUT*NB/16]
    gidx_arr = np.ascontiguousarray(np.tile(gidx16, (P // 16, 1)))

    lut3 = lut_table.reshape(OHI, P, 64).transpose(1, 0, 2)  # [o_p, o_hi, 64]
    lutg_arr = np.ascontiguousarray(lut3)

    in_maps = []
    for core in range(N_CORES):
        in_maps.append(
            {
                "xs": np.ascontiguousarray(x[core * P : (core + 1) * P]),
                "gidx": gidx_arr,
                "lutg": lutg_arr,
            }
        )
    return in_maps


def assemble_output(results):
    """results: list of 8 dicts with 'outs' [128, 16, 128] -> full [1024, 2048]."""
    parts = []
    for core in range(N_CORES):
        arr = results[core]["outs"]  # [o_p, o_hi, b]
        parts.append(np.ascontiguousarray(arr.transpose(2, 1, 0).reshape(P, OUT)))
    return np.concatenate(parts, axis=0)


def kernel_with_results(x, lut_table, mapping, **kwargs):
    nc = _program()
    in_maps = make_inputs(x, lut_table, mapping)
    res = run_bass_kernel_spmd(nc, in_maps, core_ids=list(range(N_CORES)), **kwargs)
    return assemble_output(res.results), res


def kernel(x, lut_table, mapping):
    out, _ = kernel_with_results(x, lut_table, mapping)
    return out


if __name__ == "__main__":
    rng = np.random.default_rng(0)
    x = rng.random((B_FULL, IN), dtype=np.float32)
    lut = rng.standard_normal((OUT, 64), dtype=np.float32)
    mp = rng.integers(0, IN, (OUT, NB), dtype=np.int32)
    out = kernel(x, lut, mp)
    print(out.shape, out.dtype)


# revision 34
# speedup vs baseline: 1.4033x; 1.4033x over previous
"""Trainium2 Bass kernel for nn_BaseLUTLayer (soft-LUT layer).

Math: out[b,o] = sum_k lut[o,k] * prod_j (bit_j(k) ? x[b,m(o,j)] : 1-x[b,m(o,j)])

Strategy (per core, batch-sharded 8 ways, 128 batch rows each):
  * odds transform: with w = 1-x, r = x/(1-x):
        out[b,o] = (prod_j w_j) * H,   H = successive halving of lut with
        T_new[k'] = T_lo[k'] + r_j * T_hi[k']   (6 levels, 2 DVE ops/elem)
  * layout: nodes on SBUF partitions (o_p = o % 128), free dims (k', b).
    lut tiles live per-partition (no replication); r/w values are gathered
    per (node, wire) with dma_gather using compile-time indices derived
    from `mapping` (host-known at trace time).
  * gather source: G[row(i)] = [w[:,i] (128 f32) | r[:,i] (128 f32)] built
    on-device (clamp, 1-x, reciprocal, PE transposes) and bounced via HBM.
"""

import numpy as np

import concourse.bass as bass
import concourse.mybir as mybir
from concourse import bacc
from concourse import tile
from concourse.masks import make_identity
from concourse.bass_utils import run_bass_kernel_spmd

P = 128
IN = 1024
OUT = 2048
NB = 6
B_FULL = 1024
N_CORES = 8
OHI = OUT // P  # 16
F32 = mybir.dt.float32
I16 = mybir.dt.int16
# clamp x <= 1 - 2^-18 so r = x/(1-x) <= 2^18 and r^6 stays well inside fp32
CLAMP = float(1.0 - 2.0**-18)

# chunks of the o_hi loop assigned to gpsimd instead of DVE (load balance:
# gpsimd 2-input elementwise is ~2x slower than DVE, so give it ~1/3)
GPSIMD_CHUNKS = ()


def _mult():
    return mybir.AluOpType.mult


def _add():
    return mybir.AluOpType.add


def build_program():
    nc = bacc.Bacc("TRN2", target_bir_lowering=False, debug=False)

    xs = nc.dram_tensor("xs", [P, IN], F32, kind="ExternalInput").ap()
    gidx = nc.dram_tensor("gidx", [P, OUT * NB // 16], I16, kind="ExternalInput").ap()
    lutg = nc.dram_tensor("lutg", [P, OHI, 64], F32, kind="ExternalInput").ap()
    outs = nc.dram_tensor("outs", [P, OHI, P], F32, kind="ExternalOutput").ap()

    with tile.TileContext(nc) as tc:
        with (
            tc.tile_pool(name="consts", bufs=1) as consts,
            tc.tile_pool(name="main", bufs=1) as main,
            tc.tile_pool(name="zpool", bufs=4) as zpool,
            tc.tile_pool(name="tpool", bufs=3) as tpool,
            tc.tile_pool(name="spool", bufs=2) as spool,
            tc.tile_pool(name="dram", bufs=1, space="DRAM") as dpool,
        ):
            ident = consts.tile([P, P], F32)
            make_identity(nc, ident)

            gd = dpool.tile([P * (IN // P), 2 * P], F32)
            gd_warm = gd

            gidx_sb = consts.tile([P, OUT * NB // 16], I16)
            nc.sync.dma_start(gidx_sb, gidx)
            lutg_sb = consts.tile([P, OHI, 64], F32)
            nc.sync.dma_start(lutg_sb, lutg)

            # warm up the dma_gather ucode (IRAM load) before G is ready:
            # zero gd row 0, gather it 128 times into a scratch tile
            wzt = consts.tile([1, 2 * P], F32)
            nc.gpsimd.memset(wzt, 0.0)
            nc.sync.dma_start(gd_warm[0:1, :], wzt)
            widx = consts.tile([P, 8], I16)
            nc.gpsimd.memset(widx, 0)
            warm = consts.tile([P, 1, 2 * P], F32)
            nc.gpsimd.dma_gather(
                out_ap=warm,
                in_ap=gd_warm[0:1, :],
                idxs_ap=widx,
                num_idxs=P,
                num_idxs_reg=P,
                elem_size=2 * P,
            )

            # x shard, clamped; w = 1-x; r = x * (1/w)
            xt = main.tile([P, IN], F32)
            nc.sync.dma_start(xt, xs)
            nc.vector.tensor_scalar_min(xt, xt, CLAMP)
            wt = main.tile([P, IN], F32)
            nc.vector.tensor_scalar(
                out=wt, in0=xt, scalar1=-1.0, scalar2=1.0, op0=_mult(), op1=_add()
            )
            rw = main.tile([P, IN], F32)
            rt = main.tile([P, IN], F32)
            for q in range(4):
                qs = slice(q * (IN // 4), (q + 1) * (IN // 4))
                nc.vector.reciprocal(rw[:, qs], wt[:, qs])
                nc.vector.tensor_mul(rt[:, qs], xt[:, qs], rw[:, qs])

            # transpose w/r into G rows: G[(i%128)*8 + i//128] = [w[:,i] | r[:,i]]
            gsb = main.tile([P, IN // P, 2 * P], F32)
            with tc.tile_pool(name="psum_t", bufs=2, space="PSUM") as psum_t:
                for ih in range(IN // P):
                    pw = psum_t.tile([P, P], F32, tag="pt")
                    nc.tensor.transpose(pw, wt[:, ih * P : (ih + 1) * P], ident)
                    nc.scalar.copy(gsb[:, ih, 0:P], pw)
                    pr = psum_t.tile([P, P], F32, tag="pt")
                    nc.tensor.transpose(pr, rt[:, ih * P : (ih + 1) * P], ident)
                    nc.scalar.copy(gsb[:, ih, P : 2 * P], pr)

            gd_view = gd[:].rearrange("(p h) e -> p h e", h=IN // P)
            nc.sync.dma_start(gd_view, gsb)

            # main loop over node chunks (128 nodes each)
            psum_cm = tc.tile_pool(name="psum", bufs=2, space="PSUM")
            psum = psum_cm.__enter__()
            idx_cols = NB * P // 16  # 48 idx columns per chunk

            # two-stage software pipeline: stage A (gather + monomial muls +
            # DMA pair-adds) for chunk c, then stage B (everything after the
            # DMA-adds) for chunk c-1 — keeps DVE's in-order queue from
            # stalling on the DMA-add completion.
            stash = {}

            def stage_a(c):
                z = zpool.tile([P, NB, 2 * P], F32, tag="z")
                nc.gpsimd.dma_gather(
                    out_ap=z,
                    in_ap=gd[:],
                    idxs_ap=gidx_sb[:, c * idx_cols : (c + 1) * idx_cols],
                    num_idxs=NB * P,
                    num_idxs_reg=NB * P,
                    elem_size=2 * P,
                )
                # W = prod_j w_j (DVE)
                wp = spool.tile([P, 3, P], F32, tag="wp")
                nc.vector.tensor_mul(wp, z[:, 1:6:2, 0:P], z[:, 0:5:2, 0:P])
                wq = spool.tile([P, P], F32, tag="wq")
                nc.vector.tensor_mul(wq, wp[:, 0, :], wp[:, 1, :])
                nc.vector.tensor_mul(wq, wq, wp[:, 2, :])
                # level 1 (DVE): T1[k'] = lutc0|lutc5 tables: lut lo/hi halves
                t1 = tpool.tile([P, 32, P], F32, tag="t1")
                nc.vector.tensor_mul(
                    t1,
                    z[:, 5, P : 2 * P][:, None, :].broadcast_to([P, 32, P]),
                    lutg_sb[:, c, 32:64][:, :, None].broadcast_to([P, 32, P]),
                )
                nc.vector.tensor_add(
                    t1, t1, lutg_sb[:, c, 0:32][:, :, None].broadcast_to([P, 32, P])
                )
                # level 2 prod (DVE): prod2 = r4 * T1_hi
                prod2 = tpool.tile([P, 16, P], F32, tag="pr16")
                nc.vector.tensor_mul(
                    prod2,
                    z[:, 4, P : 2 * P][:, None, :].broadcast_to([P, 16, P]),
                    t1[:, 16:32, :],
                )
                # L2 on TensorE: acc[0:2048] = T1_lo + prod2 (identity MMs);
                # close regions [1024:2048] (R2,R3) — read-only afterwards
                acc = psum.tile([P, 16 * P], F32, tag="pacc")
                t1f = t1[:].rearrange("p a b -> p (a b)")
                p2f = prod2[:].rearrange("p a b -> p (a b)")
                for s in range(4):
                    sl = slice(s * 512, (s + 1) * 512)
                    nc.tensor.matmul(acc[:, sl], ident, t1f[:, sl], start=True, stop=False)
                    nc.tensor.matmul(acc[:, sl], ident, p2f[:, sl], start=False, stop=(s >= 2))
                stash[c] = (z, acc, wq)

            def stage_b(c):
                z, acc, wq = stash.pop(c)

                # level 3 (j=3, h=8): prod3 = r3*acc[8:16] (DVE, PSUM src);
                # acc[0:8] += prod3 (PE), closing R0,R1
                pn8 = tpool.tile([P, 8, P], F32, tag="pr8")
                nc.vector.tensor_mul(
                    pn8,
                    z[:, 3, P : 2 * P][:, None, :].broadcast_to([P, 8, P]),
                    acc[:, 8 * P : 16 * P].rearrange("p (a b) -> p a b", b=P),
                )
                pn8f = pn8[:].rearrange("p a b -> p (a b)")
                for s in range(2):
                    sl = slice(s * 512, (s + 1) * 512)
                    nc.tensor.matmul(acc[:, sl], ident, pn8f[:, sl], start=False, stop=True)

                # level 4 (j=2, h=4) on DVE from closed PSUM
                pn4 = tpool.tile([P, 4, P], F32, tag="pr4")
                nc.vector.tensor_mul(
                    pn4,
                    z[:, 2, P : 2 * P][:, None, :].broadcast_to([P, 4, P]),
                    acc[:, 4 * P : 8 * P].rearrange("p (a b) -> p a b", b=P),
                )
                t4 = tpool.tile([P, 4, P], F32, tag="t4")
                nc.vector.tensor_add(
                    t4, pn4, acc[:, 0 : 4 * P].rearrange("p (a b) -> p a b", b=P)
                )

                # level 5 (j=1, h=2)
                pn2 = tpool.tile([P, 2, P], F32, tag="pr2")
                nc.vector.tensor_mul(
                    pn2,
                    z[:, 1, P : 2 * P][:, None, :].broadcast_to([P, 2, P]),
                    t4[:, 2:4, :],
                )
                t5 = tpool.tile([P, 2, P], F32, tag="t5")
                nc.vector.tensor_add(t5, pn2, t4[:, 0:2, :])

                # level 6 (j=0, h=1)
                pn1 = tpool.tile([P, 1, P], F32, tag="pr1")
                nc.vector.tensor_mul(
                    pn1,
                    z[:, 0, P : 2 * P][:, None, :].broadcast_to([P, 1, P]),
                    t5[:, 1:2, :],
                )
                t6 = tpool.tile([P, 1, P], F32, tag="t6")
                nc.vector.tensor_add(t6, pn1, t5[:, 0:1, :])

                ot = spool.tile([P, P], F32, tag="ot")
                nc.vector.tensor_mul(ot, t6[:, 0, :], wq)
                nc.sync.dma_start(outs[:, c, :], ot)

            for c in range(OHI + 1):
                if c < OHI:
                    stage_a(c)
                if c >= 1:
                    stage_b(c - 1)
            psum_cm.__exit__(None, None, None)

    # Bacc passes: event-sem generation (multi-wait lowering), auto library
    # loads for dma_gather, extended-InstISA byte packing, ...
    nc.compile()
    return nc


_CACHE: dict = {}


def _program():
    if "nc" not in _CACHE:
        _CACHE["nc"] = build_program()
    return _CACHE["nc"]


def make_inputs(x, lut_table, mapping):
    """Host-side input prep: shard x by batch, encode mapping as gather
    indices, split lut into node-on-partition lo/hi tiles."""
    x = np.ascontiguousarray(x, dtype=np.float32)
    lut_table = np.ascontiguousarray(lut_table, dtype=np.float32)
    mapping = np.asarray(mapping)

    # gather row of source column i: G row (i%128)*8 + i//128
    m3 = mapping.reshape(OHI, P, NB)  # [o_hi, o_p, j]
    rows = (m3 % P) * (IN // P) + (m3 // P)
    # t = (o_hi*NB + j)*128 + o_p  ->  order (o_hi, j, o_p)
    tvals = np.transpose(rows, (0, 2, 1)).reshape(-1)
    gidx16 = tvals.reshape(-1, 16).T.astype(np.int16)  # [16, OUT*NB/16]
    gidx_arr = np.ascontiguousarray(np.tile(gidx16, (P // 16, 1)))

    lut3 = lut_table.reshape(OHI, P, 64).transpose(1, 0, 2)  # [o_p, o_hi, 64]
    lutg_arr = np.ascontiguousarray(lut3)

    in_maps = []
    for core in range(N_CORES):
        in_maps.append(
            {
                "xs": np.ascontiguousarray(x[core * P : (core + 1) * P]),
                "gidx": gidx_arr,
                "lutg": lutg_arr,
            }
        )
    return in_maps


def assemble_output(results):
    """results: list of 8 dicts with 'outs' [128, 16, 128] -> full [1024, 2048]."""
    parts = []
    for core in range(N_CORES):
        arr = results[core]["outs"]  # [o_p, o_hi, b]
        parts.append(np.ascontiguousarray(arr.transpose(2, 1, 0).reshape(P, OUT)))
    return np.concatenate(parts, axis=0)


def kernel_with_results(x, lut_table, mapping, **kwargs):
    nc = _program()
    in_maps = make_inputs(x, lut_table, mapping)
    res = run_bass_kernel_spmd(nc, in_maps, core_ids=list(range(N_CORES)), **kwargs)
    return assemble_output(res.results), res


def kernel(x, lut_table, mapping):
    out, _ = kernel_with_results(x, lut_table, mapping)
    return out


if __name__ == "__main__":
    rng = np.random.default_rng(0)
    x = rng.random((B_FULL, IN), dtype=np.float32)
    lut = rng.standard_normal((OUT, 64), dtype=np.float32)
    mp = rng.integers(0, IN, (OUT, NB), dtype=np.int32)
    out = kernel(x, lut, mp)
    print(out.shape, out.dtype)
